# revision 1
# baseline (speedup 1.0000x reference)
"""CRF Viterbi decode (forward max-plus scan + backpointer backtrace + one-hot)
on 8 Trainium2 NeuronCores, data-parallel over the batch axis.

Host contract: kernel(x[256,1024,128] f32, transitions[128,128] f32,
seq_lens[256] i32) -> one_hot(tags)[256,1024,128] f32, bit-matching the jax
reference (first-index argmax tie-breaking).

Per-core layout (BLOC=32 batches as SBUF partitions 0..31, C=128 classes):
  forward step t: for each batch b, scores = Tt + broadcast(alpha[b,:]) is
  computed by a fused tensor_tensor_reduce (add + running max -> m) on DVE;
  the PE broadcasts alpha rows into PSUM via K=1 ones-matmuls (exact fp32);
  max_index extracts the first-argmax backpointer column; backpointers are
  transposed (PE) into [b, j] rows and streamed to DRAM. Sequence-length
  masking is data-driven via copy_predicated so one SPMD program serves all
  cores; lanes are globally sorted by seq_len so a phase schedule (shrinking
  active-lane count) compacts the work.
"""

import sys

sys.path.insert(0, "/opt/trn_rl_repo")

import numpy as np

B, T, C = 256, 1024, 128
NCORES = 8
BLOC = B // NCORES  # 32

_prog_cache = {}


def _assign_lanes(seq_lens: np.ndarray) -> np.ndarray:
    """Snake-deal batches (sorted by seq_len desc) to cores -> [NCORES, BLOC]."""
    order = np.argsort(-seq_lens, kind="stable")
    lanes = [[] for _ in range(NCORES)]
    for i, b in enumerate(order):
        r, k = divmod(i, NCORES)
        c = k if r % 2 == 0 else NCORES - 1 - k
        lanes[c].append(int(b))
    return np.array(lanes, dtype=np.int64)


def _phase_schedule(seq_lens, lanes, ct, t_len, bloc):
    """Per chunk of CT steps, the max (over cores) active-lane count, merged
    into maximal runs [(chunk_lo, chunk_hi, n), ...]."""
    nch = (t_len + ct - 1) // ct
    L = seq_lens[lanes]  # [NCORES, BLOC] descending per row
    ns = []
    for cidx in range(nch):
        t0 = cidx * ct
        n = int(max((row > t0).sum() for row in L))
        n = max(n, 1)
        # quantize up to a few levels so phases span many chunks (keeps the
        # traced instruction count down; For_i loops the chunks of a phase)
        q = bloc // 4 if bloc >= 4 else 1
        n = min(bloc, ((n + q - 1) // q) * q)
        ns.append(n)
    phases = []
    lo = 0
    for cidx in range(1, nch + 1):
        if cidx == nch or ns[cidx] != ns[lo]:
            phases.append((lo, cidx, ns[lo]))
            lo = cidx
    return phases


def build_program(t_len=T, ct=64, bloc=BLOC, phases=None, gs=8, trn="TRN2",
                  num_devices=NCORES):
    """Build the SPMD bass program. Returns (nc, meta)."""
    import concourse.bass as bass
    import concourse.bacc as bacc
    import concourse.mybir as mybir
    from concourse import tile

    f32 = mybir.dt.float32
    u32 = mybir.dt.uint32
    i8 = mybir.dt.int8
    Alu = mybir.AluOpType
    nch = (t_len + ct - 1) // ct
    assert nch * ct == t_len
    if phases is None:
        phases = [(0, nch, bloc)]

    nc = bacc.Bacc(trn, target_bir_lowering=False, debug=False,
                   num_devices=num_devices)

    TC = t_len * C
    x_d = nc.dram_tensor("x", [bloc, TC], f32, kind="ExternalInput").ap()
    tt_d = nc.dram_tensor("tt", [C, C], f32, kind="ExternalInput").ap()
    esel_d = nc.dram_tensor("esel", [bloc, bloc * C], f32, kind="ExternalInput").ap()
    ident_d = nc.dram_tensor("ident", [C, C], f32, kind="ExternalInput").ap()
    iota_d = nc.dram_tensor("iota", [bloc, C], f32, kind="ExternalInput").ap()
    mask_d = nc.dram_tensor("mask", [bloc, t_len], i8, kind="ExternalInput").ap()
    bp_d = nc.dram_tensor("bp", [bloc, TC], f32)  # internal DRAM scratch
    out_d = nc.dram_tensor("out", [bloc, TC], f32, kind="ExternalOutput").ap()

    # persistent SBUF
    tt_sb = nc.alloc_sbuf_tensor("tt_sb", [C, C], f32).ap()
    esel_sb = nc.alloc_sbuf_tensor("esel_sb", [bloc, bloc * C], f32).ap()
    ident_sb = nc.alloc_sbuf_tensor("ident_sb", [C, C], f32).ap()
    iota_sb = nc.alloc_sbuf_tensor("iota_sb", [bloc, C], f32).ap()
    alpha = nc.alloc_sbuf_tensor("alpha", [bloc, C], f32).ap()
    m_t = nc.alloc_sbuf_tensor("m_t", [C, bloc], f32)
    bpu_t = nc.alloc_sbuf_tensor("bpu_t", [C, 8 * bloc], u32)
    bpf = nc.alloc_sbuf_tensor("bpf", [C, bloc], f32).ap()
    tagv = nc.alloc_sbuf_tensor("tagv", [bloc, t_len], f32).ap()
    xr = [nc.alloc_sbuf_tensor(f"xr{p}", [bloc, ct * C], f32).ap() for p in (0, 1)]
    br = [nc.alloc_sbuf_tensor(f"br{p}", [bloc, ct * C], f32).ap() for p in (0, 1)]
    mr = [nc.alloc_sbuf_tensor(f"mr{p}", [bloc, ct], i8).ap() for p in (0, 1)]

    m_ap = m_t.ap()
    bpu_ap = bpu_t.ap()

    def bcast(ap, dim, n):
        """Insert a step-0 (broadcast) dim of size n at position dim."""
        a = ap[tuple(slice(None) for _ in ap.shape)]
        a.ap.insert(dim, [0, n])
        return a

    def m8(b):
        # m_t[:, b] broadcast to free size 8 (for max_index's in_max)
        return bass.AP(m_t, b, [[bloc, C], [0, 8]])

    def bpu_col0():
        # column 0 of each 8-wide slot: [C, bloc] u32 view
        return bass.AP(bpu_t, 0, [[8 * bloc, C], [8, bloc]])

    def mask_col(ring, s, n):
        # mask ring column s broadcast along free C -> [n, C]
        return bass.AP(ring.tensor, ring.offset + s, [[ct, n], [0, C]])

    with tile.TileContext(nc) as tc:
        with (
            tc.tile_pool(name="psA", bufs=2, space="PSUM") as psA,
            tc.tile_pool(name="psS", bufs=2, space="PSUM") as psS,
            tc.tile_pool(name="sbA", bufs=2) as sbA,
            tc.tile_pool(name="sc", bufs=4) as scp,
        ):
            # one-time loads
            nc.sync.dma_start(out=tt_sb, in_=tt_d)
            nc.sync.dma_start(out=esel_sb, in_=esel_d)
            nc.sync.dma_start(out=ident_sb, in_=ident_d)
            nc.sync.dma_start(out=iota_sb, in_=iota_d)
            nc.sync.dma_start(out=alpha, in_=x_d[:, 0:C])  # alpha0 = x[:,0,:]

            def chunk_body(iv, p, n, first_skip):
                """Forward chunk: steps s=0..ct-1 of chunk iv (parity p).
                n = active lanes. first_skip: skip s==0 (t==0) in chunk 0."""
                nc.sync.dma_start(out=xr[p], in_=x_d[:, bass.ds(iv * ct * C, ct * C)])
                nc.sync.dma_start(out=mr[p], in_=mask_d[:, bass.ds(iv * ct, ct)])
                # identity prefill of bp ring (masked steps keep iota rows)
                nc.scalar.copy(out=br[p].rearrange("b (s c) -> b s c", c=C),
                               in_=bcast(iota_sb, 1, ct))
                ng = (n + gs - 1) // gs
                for s in range(ct):
                    if first_skip and s == 0:
                        continue
                    for g in range(ng):
                        b0, b1 = g * gs, min(n, (g + 1) * gs)
                        nb = b1 - b0
                        abuf = psA.tile([C, gs * C], f32, tag="abuf")
                        for k in range(nb):
                            b = b0 + k
                            nc.tensor.matmul(
                                abuf[:, k * C:(k + 1) * C],
                                esel_sb[:, b * C:(b + 1) * C], alpha,
                                start=True, stop=True,
                            )
                        asb = sbA.tile([C, gs * C], f32, tag="asb")
                        nc.scalar.copy(out=asb[:, 0:nb * C], in_=abuf[:, 0:nb * C])
                        sc = scp.tile([C, gs * C], f32, tag="sc")
                        nc.vector.tensor_tensor(
                            out=sc[:, 0:nb * C].rearrange("j (k c) -> j k c", c=C),
                            in0=bcast(tt_sb, 1, nb),
                            in1=asb[:, 0:nb * C].rearrange("j (k c) -> j k c", c=C),
                            op=Alu.add)
                        nc.vector.tensor_reduce(
                            out=m_ap[:, b0:b1],
                            in_=sc[:, 0:nb * C].rearrange("j (k c) -> j k c", c=C),
                            axis=mybir.AxisListType.X, op=Alu.max)
                        for k in range(nb):
                            b = b0 + k
                            nc.vector.max_index(
                                out=bpu_ap[:, b * 8:(b + 1) * 8],
                                in_max=m8(b), in_values=sc[:, k * C:(k + 1) * C],
                            )
                    # bp column extract + transposes + state update
                    nc.vector.tensor_copy(out=bpf[:, 0:n], in_=bass.AP(bpu_t, 0, [[8 * bloc, C], [8, n]]))
                    tpA = psS.tile([bloc, C], f32, tag="tpA")
                    nc.tensor.transpose(tpA[:], m_ap, ident_sb)
                    tpB = psS.tile([bloc, C], f32, tag="tpB")
                    nc.tensor.transpose(tpB[:], bpf, ident_sb)
                    sc2 = scp.tile([bloc, C], f32, tag="sc2")
                    nc.vector.tensor_tensor(
                        out=sc2[0:n, :], in0=tpA[0:n, :],
                        in1=xr[p][0:n, s * C:(s + 1) * C], op=Alu.add)
                    nc.vector.copy_predicated(
                        out=alpha[0:n, :], mask=mask_col(mr[p], s, n),
                        data=sc2[0:n, :])
                    nc.vector.copy_predicated(
                        out=br[p][0:n, s * C:(s + 1) * C],
                        mask=mask_col(mr[p], s, n), data=tpB[0:n, :])
                nc.sync.dma_start(out=bp_d.ap()[:, bass.ds(iv * ct * C, ct * C)],
                                  in_=br[p])

            # ---- forward phases ----
            for (lo, hi, n) in phases:
                def mk(nn, is_first):
                    def body2(iv0, unroll):
                        for u in range(unroll):
                            chunk_body(iv0 + u, u % 2, nn,
                                       first_skip=(is_first and u == 0))
                    return body2
                if lo == 0:
                    # chunk 0 traced alone (skips t=0)
                    tc.For_i_unrolled_general(
                        start=0, end=1, step=1,
                        unrollable_body=mk(n, True), max_unroll=1)
                    if hi > 1:
                        tc.For_i_unrolled_general(
                            start=1, end=hi, step=1,
                            unrollable_body=mk(n, False), max_unroll=1)
                else:
                    tc.For_i_unrolled_general(
                        start=lo, end=hi, step=1,
                        unrollable_body=mk(n, False), max_unroll=1)

            # ---- last tag ----
            mx = scp.tile([bloc, 1], f32, tag="mx")
            nc.vector.tensor_reduce(out=mx[:], in_=alpha, axis=mybir.AxisListType.X,
                                    op=Alu.max)
            lt8 = scp.tile([bloc, 8], u32, tag="lt8")
            mx8 = bass.AP(mx.tensor, mx.offset, [[mx.ap.to_list()[0][0], bloc], [0, 8]])
            nc.vector.max_index(out=lt8[:], in_max=mx8, in_values=alpha)
            nc.vector.tensor_copy(out=tagv[:, t_len - 1:t_len], in_=lt8[:, 0:1])

            # ---- backtrace + one-hot (static chunk loop, reversed) ----
            for cc in range(nch - 1, -1, -1):
                p = cc % 2
                nc.sync.dma_start(out=xr[p], in_=bp_d.ap()[:, cc * ct * C:(cc + 1) * ct * C])
                for s in range(ct - 1, -1, -1):
                    u = cc * ct + s
                    if u == 0:
                        continue
                    oh = scp.tile([bloc, C], f32, tag="oh")
                    nc.vector.tensor_scalar(
                        out=oh[:], in0=iota_sb, scalar1=tagv[:, u:u + 1],
                        scalar2=None, op0=Alu.is_equal)
                    dot = scp.tile([bloc, C], f32, tag="dot")
                    nc.vector.scalar_tensor_tensor(
                        out=dot[:], in0=oh[:], scalar=1.0,
                        in1=xr[p][:, s * C:(s + 1) * C],
                        op0=Alu.mult, op1=Alu.mult,
                        accum_out=tagv[:, u - 1:u])
                # one-hot emit for chunk cc (tags for its steps are final)
                ohc = br[p]
                nc.vector.tensor_tensor(
                    out=ohc.rearrange("b (s c) -> b s c", c=C),
                    in0=bcast(iota_sb, 1, ct),
                    in1=bcast(tagv[:, cc * ct:(cc + 1) * ct], 2, C),
                    op=Alu.is_equal)
                nc.sync.dma_start(out=out_d[:, cc * ct * C:(cc + 1) * ct * C],
                                  in_=ohc)

    nc.compile()
    return nc


def _host_inputs(x, transitions, seq_lens, lanes, t_len=T, bloc=BLOC):
    """Build per-core input maps."""
    tt = np.ascontiguousarray(transitions.T).astype(np.float32)
    esel = np.zeros((bloc, bloc * C), np.float32)
    for b in range(bloc):
        esel[b, b * C:(b + 1) * C] = 1.0
    ident = np.eye(C, dtype=np.float32)
    iota = np.tile(np.arange(C, dtype=np.float32), (bloc, 1))
    in_maps = []
    for c in range(lanes.shape[0]):
        lx = x[lanes[c]][:, :t_len, :].reshape(bloc, t_len * C).astype(np.float32)
        L = seq_lens[lanes[c]].astype(np.int64)
        tgrid = np.arange(t_len)[None, :]
        mask = (tgrid < L[:, None]).astype(np.int8)  # active at step t: t < L
        in_maps.append({
            "x": np.ascontiguousarray(lx),
            "tt": tt, "esel": esel, "ident": ident, "iota": iota,
            "mask": np.ascontiguousarray(mask),
        })
    return in_maps


TRACE = False
LAST_RESULT = None


def kernel(x, transitions, seq_lens):
    global LAST_RESULT
    from concourse.bass_utils import run_bass_kernel_spmd

    x = np.asarray(x, dtype=np.float32)
    transitions = np.asarray(transitions, dtype=np.float32)
    seq_lens = np.asarray(seq_lens)
    lanes = _assign_lanes(seq_lens)
    ct = 64
    phases = _phase_schedule(seq_lens, lanes, ct, T, BLOC)
    key = tuple(phases)
    if key not in _prog_cache:
        _prog_cache[key] = build_program(T, ct, BLOC, phases)
    nc = _prog_cache[key]
    in_maps = _host_inputs(x, transitions, seq_lens, lanes)
    res = run_bass_kernel_spmd(nc, in_maps, list(range(NCORES)), trace=TRACE)
    LAST_RESULT = res
    out = np.empty((B, T, C), np.float32)
    for c in range(NCORES):
        out[lanes[c]] = res.results[c]["out"].reshape(BLOC, T, C)
    return out

